# revision 6
# baseline (speedup 1.0000x reference)
"""Feedforward SNN (Linear -> LIF) x2 kernel for Trainium2, 8-core data parallel.

Per-core plan (B is sharded 8 ways, BL=32 samples/core):
  - Layer-1 currents for ALL timesteps are one big matmul (x does not depend
    on recurrent state):  Cur1[h1, (t,b)] = W1 @ x^T.
  - LIF-1 scan runs over t on [128, HC1*32] tiles (partition = h1 % 128,
    free = (h1chunk, b)), 3 DVE ops/step via fused scalar_tensor_tensor.
  - Layer-2 currents are also batched per t-block: spk1 for a block of
    T_NB steps feeds one fat matmul vs W2^T.
  - LIF-2 scan the same way (2 DVE ops/step; spikes never materialized
    except the final step).
  - W1^T is built on-chip (PE transpose) and kept resident; W2^T is built
    once into a DRAM scratch and streamed per t-block.

All matmuls fp32 (4 cyc/row on PE) to stay inside the fp32 envelope of the
reference (the heaviside makes lower precision diverge chaotically).
"""

import os
import sys

import numpy as np

for _p in ("/opt/trn_rl_repo", "/root/.axon_site/_ro/trn_rl_repo"):
    if os.path.isdir(_p) and _p not in sys.path:
        sys.path.insert(0, _p)

import concourse.bass as bass  # noqa: E402
import concourse.mybir as mybir  # noqa: E402
import concourse.tile as tile  # noqa: E402
from concourse import bacc  # noqa: E402
from concourse.bass_utils import run_bass_kernel_spmd  # noqa: E402
from concourse.masks import make_identity  # noqa: E402

F32 = mybir.dt.float32
ALU = mybir.AluOpType
AF = mybir.ActivationFunctionType

BETA = 0.9
THR = 1.0

# Full-problem dims
B_FULL, T_FULL, D_FULL, H1_FULL, H2_FULL = 256, 64, 1024, 2048, 2048
N_CORES = 8
BL = B_FULL // N_CORES  # 32


def build_snn(T=T_FULL, D=D_FULL, H1=H1_FULL, H2=H2_FULL, T_NB=8,
              mm_dtype=F32):
    """Build the single-core Bass program (identical across the 8 cores)."""
    P = 128
    KC1 = D // P        # k-chunks of matmul1
    HC1 = H1 // P       # h1 chunks (= k-chunks of matmul2)
    HC2 = H2 // P       # h2 chunks
    NNB = T // T_NB     # number of t-blocks
    SUB = min(4, T_NB)  # steps per cur sub-tile
    NSUB = T_NB // SUB
    TG = min(4, T_NB)   # timesteps per x-stage tile (128 = TG*32 partitions)
    NTG = T_NB // TG
    MCQ = 4             # h2 chunks per matmul2 psum group
    HCQ = 4             # h1 chunks per matmul1 psum group

    assert T % T_NB == 0 and T_NB % SUB == 0 and TG * 32 == 128
    assert HC2 % MCQ == 0 and HC1 % HCQ == 0

    nc = bacc.Bacc("TRN2", target_bir_lowering=False, debug=False)

    x_d = nc.dram_tensor("x", [BL, T, D], F32, kind="ExternalInput")
    w1_d = nc.dram_tensor("W1", [H1, D], F32, kind="ExternalInput")
    b1_d = nc.dram_tensor("b1", [H1], F32, kind="ExternalInput")
    w2_d = nc.dram_tensor("W2", [H2, H1], F32, kind="ExternalInput")
    b2_d = nc.dram_tensor("b2", [H2], F32, kind="ExternalInput")

    spk2_d = nc.dram_tensor("spk2", [BL, H2], F32, kind="ExternalOutput")
    mem1_d = nc.dram_tensor("mem1", [BL, H1], F32, kind="ExternalOutput")
    mem2_d = nc.dram_tensor("mem2", [BL, H2], F32, kind="ExternalOutput")

    # W2^T staged through DRAM scratch: [kc, p, h2] = W2[h2, kc*128+p]
    w2t_d = nc.dram_tensor("w2t_scr", [HC1, P, H2], F32, kind="Internal")

    x_ap = x_d.ap()
    w1_ap = w1_d.ap()
    w2_ap = w2_d.ap()

    with tile.TileContext(nc) as tc:
        from contextlib import ExitStack
        ctx = ExitStack()
        with ctx:
            const = ctx.enter_context(tc.tile_pool(name="const", bufs=1))
            wstage = ctx.enter_context(tc.tile_pool(name="wstage", bufs=2))
            small = ctx.enter_context(tc.tile_pool(name="small", bufs=4))
            xstage = ctx.enter_context(tc.tile_pool(name="xstage", bufs=2))
            xtp = ctx.enter_context(tc.tile_pool(name="xtp", bufs=2))
            curp = ctx.enter_context(tc.tile_pool(name="curp", bufs=3))
            spk1p = ctx.enter_context(tc.tile_pool(name="spk1p", bufs=1))
            w2tp = ctx.enter_context(tc.tile_pool(name="w2tp", bufs=2))
            statep = ctx.enter_context(tc.tile_pool(name="statep", bufs=2))
            negzp = ctx.enter_context(tc.tile_pool(name="negzp", bufs=1))
            outp = ctx.enter_context(tc.tile_pool(name="outp", bufs=4))
            tpsum = ctx.enter_context(
                tc.tile_pool(name="tpsum", bufs=2, space="PSUM"))
            mpsum = ctx.enter_context(
                tc.tile_pool(name="mpsum", bufs=6, space="PSUM"))

            ident = const.tile([P, P], F32, name="ident")
            make_identity(nc, ident)

            # biases laid out [p, chunk]: b[h] at (h % 128, h // 128)
            b1s = const.tile([P, HC1], F32, name="b1s")
            nc.sync.dma_start(b1s[:], b1_d.ap().rearrange("(c p) -> p c", p=P))
            b2s = const.tile([P, HC2], F32, name="b2s")
            nc.sync.dma_start(b2s[:], b2_d.ap().rearrange("(c p) -> p c", p=P))

            # ---------------- phase 0a: W1 -> W1T resident -----------------
            # W1T[p=d%128, kc, h1]  (lhsT tiles for matmul1)
            w1t = const.tile([P, KC1, H1], F32, name="w1t")
            cp_i = 0  # alternate copy engine to split ACT/DVE load
            for hc in range(HC1):
                for kcg in range(0, KC1, 8):
                    kcn = min(8, KC1 - kcg)
                    st = wstage.tile([P, 8 * P], F32, tag="wstage", name="w1st")
                    nc.sync.dma_start(
                        st[:, :kcn * P],
                        w1_ap[hc * P:(hc + 1) * P, kcg * P:(kcg + kcn) * P])
                    for k in range(kcn):
                        ps = tpsum.tile([P, P], F32, tag="tp", name="w1ps")
                        nc.tensor.transpose(
                            ps[:], st[:, k * P:(k + 1) * P], ident[:])
                        dst = w1t[:, kcg + k, hc * P:(hc + 1) * P]
                        if cp_i % 2 == 0:
                            nc.scalar.activation(dst, ps[:], AF.Copy)
                        else:
                            nc.vector.tensor_copy(dst, ps[:])
                        cp_i += 1

            # ---------------- phase 0b: W2 -> W2T scratch ------------------
            for mc in range(HC2):
                for kcg in range(0, HC1, 8):
                    kcn = min(8, HC1 - kcg)
                    st = wstage.tile([P, 8 * P], F32, tag="wstage", name="w2st")
                    nc.sync.dma_start(
                        st[:, :kcn * P],
                        w2_ap[mc * P:(mc + 1) * P, kcg * P:(kcg + kcn) * P])
                    for k in range(kcn):
                        ps = tpsum.tile([P, P], F32, tag="tp", name="w2ps")
                        nc.tensor.transpose(
                            ps[:], st[:, k * P:(k + 1) * P], ident[:])
                        sb = small.tile([P, P], F32, tag="w2cp", name="w2cp")
                        if cp_i % 2 == 0:
                            nc.scalar.activation(sb[:], ps[:], AF.Copy)
                        else:
                            nc.vector.tensor_copy(sb[:], ps[:])
                        cp_i += 1
                        nc.sync.dma_start(
                            w2t_d.ap()[kcg + k, :, mc * P:(mc + 1) * P], sb[:])

            # ---------------- initial LIF state ----------------------------
            mem1_cur = statep.tile([P, HC1, 32], F32, tag="mem1", name="mem1_0")
            nc.vector.memset(mem1_cur[:], 0.0)
            mem2_cur = statep.tile([P, HC2, 32], F32, tag="mem2", name="mem2_0")
            nc.vector.memset(mem2_cur[:], 0.0)
            spk2_fin = const.tile([P, HC2, 32], F32, name="spk2_fin")

            def _mm(ap):
                return ap if mm_dtype == F32 else ap.bitcast(mm_dtype)

            # ---------------- main t-block pipeline -------------------------
            for nb in range(NNB):
                t0 = nb * T_NB

                # -- x load + transpose: xT[p=d%128, kc, (t,b)] --------------
                xt = xtp.tile([P, KC1, T_NB * 32], F32, tag="xt", name="xt")
                for tg in range(NTG):
                    xs = xstage.tile([P, D], F32, tag="xs", name="xs")
                    for tr in range(TG):
                        t = t0 + tg * TG + tr
                        nc.sync.dma_start(
                            xs[tr * 32:(tr + 1) * 32, :], x_ap[:, t, :])
                    for kc in range(KC1):
                        ps = tpsum.tile([P, P], F32, tag="tp", name="xtps")
                        nc.tensor.transpose(
                            ps[:], xs[:, kc * P:(kc + 1) * P], ident[:])
                        dst = xt[:, kc, tg * P:(tg + 1) * P]
                        if cp_i % 2 == 0:
                            nc.scalar.activation(dst, ps[:], AF.Copy)
                        else:
                            nc.vector.tensor_copy(dst, ps[:])
                        cp_i += 1

                # -- matmul1: cur1[(t,hc,b)] = W1 @ x^T + b1 -----------------
                cur1_subs = []
                for s in range(NSUB):
                    c = curp.tile([P, SUB, HC1, 32], F32, tag="cur1",
                                  name="cur1")
                    cur1_subs.append(c)
                for hq in range(HC1 // HCQ):
                    pss = []
                    for i in range(HCQ):
                        ps = mpsum.tile([P, T_NB * 32], F32, tag="mm",
                                        name="mm1ps")
                        pss.append(ps)
                    for kc in range(KC1):
                        rhs = _mm(xt[:, kc, :])
                        for i in range(HCQ):
                            hc = hq * HCQ + i
                            lhsT = _mm(w1t[:, kc, hc * P:(hc + 1) * P])
                            nc.tensor.matmul(pss[i][:], lhsT, rhs,
                                             start=(kc == 0),
                                             stop=(kc == KC1 - 1))
                    for i in range(HCQ):
                        hc = hq * HCQ + i
                        psv = pss[i].rearrange("p (t b) -> p t b", b=32)
                        for s in range(NSUB):
                            nc.scalar.activation(
                                cur1_subs[s][:, :, hc, :],
                                psv[:, s * SUB:(s + 1) * SUB, :],
                                AF.Identity, bias=b1s[:, hc:hc + 1])

                # -- scan1: T_NB steps; spikes into spk1[(kc,t,b)] -----------
                spk1 = spk1p.tile([P, HC1, T_NB * 32], F32, tag="spk1",
                                  name="spk1")
                for tr in range(T_NB):
                    cur_t = cur1_subs[tr // SUB][:, tr % SUB]  # [P, HC1, 32]
                    negz = negzp.tile([P, HC1, 32], F32, tag="negz1",
                                      name="negz1")
                    nc.vector.scalar_tensor_tensor(
                        negz[:], mem1_cur[:], THR, cur_t,
                        ALU.is_gt, ALU.subtract)
                    mem1_new = statep.tile([P, HC1, 32], F32, tag="mem1",
                                           name="mem1")
                    nc.vector.scalar_tensor_tensor(
                        mem1_new[:], mem1_cur[:], BETA, negz[:],
                        ALU.mult, ALU.subtract)
                    mem1_cur = mem1_new
                    # spike of step t is thresholded POST-update membrane
                    nc.vector.tensor_scalar(
                        spk1[:, :, tr * 32:(tr + 1) * 32], mem1_cur[:],
                        THR, None, ALU.is_gt)

                # -- matmul2: cur2[(t,mc,b)] = W2 @ spk1^T + b2 --------------
                cur2_subs = []
                for s in range(NSUB):
                    c = curp.tile([P, SUB, HC2, 32], F32, tag="cur2",
                                  name="cur2")
                    cur2_subs.append(c)
                for mq in range(HC2 // MCQ):
                    pss = []
                    for i in range(MCQ):
                        ps = mpsum.tile([P, T_NB * 32], F32, tag="mm",
                                        name="mm2ps")
                        pss.append(ps)
                    for kc in range(HC1):
                        wt = w2tp.tile([P, MCQ * P], F32, tag="w2t",
                                       name="w2t")
                        nc.sync.dma_start(
                            wt[:],
                            w2t_d.ap()[kc, :, mq * MCQ * P:(mq + 1) * MCQ * P])
                        rhs = _mm(spk1[:, kc, :])
                        for i in range(MCQ):
                            lhsT = _mm(wt[:, i * P:(i + 1) * P])
                            nc.tensor.matmul(pss[i][:], lhsT, rhs,
                                             start=(kc == 0),
                                             stop=(kc == HC1 - 1))
                    for i in range(MCQ):
                        mc = mq * MCQ + i
                        psv = pss[i].rearrange("p (t b) -> p t b", b=32)
                        for s in range(NSUB):
                            nc.scalar.activation(
                                cur2_subs[s][:, :, mc, :],
                                psv[:, s * SUB:(s + 1) * SUB, :],
                                AF.Identity, bias=b2s[:, mc:mc + 1])

                # -- scan2: T_NB steps (no spike buffer needed) --------------
                for tr in range(T_NB):
                    t = t0 + tr
                    cur_t = cur2_subs[tr // SUB][:, tr % SUB]
                    negz = negzp.tile([P, HC2, 32], F32, tag="negz2",
                                      name="negz2")
                    nc.vector.scalar_tensor_tensor(
                        negz[:], mem2_cur[:], THR, cur_t,
                        ALU.is_gt, ALU.subtract)
                    mem2_new = statep.tile([P, HC2, 32], F32, tag="mem2",
                                           name="mem2")
                    nc.vector.scalar_tensor_tensor(
                        mem2_new[:], mem2_cur[:], BETA, negz[:],
                        ALU.mult, ALU.subtract)
                    mem2_cur = mem2_new
                    if t == T - 1:
                        nc.vector.tensor_scalar(
                            spk2_fin[:], mem2_cur[:], THR, None, ALU.is_gt)

            # ---------------- outputs: unscramble [p,(hc,b)] -> [b,h] -------
            def emit_out(state, nch, out_d):
                for hc in range(nch):
                    ps = tpsum.tile([32, P], F32, tag="tp", name="ops")
                    nc.tensor.transpose(ps[:], state[:, hc, :], ident[:])
                    sb = outp.tile([32, P], F32, tag="osb", name="osb")
                    nc.scalar.activation(sb[:], ps[:], AF.Copy)
                    nc.sync.dma_start(
                        out_d.ap()[:, hc * P:(hc + 1) * P], sb[:])

            emit_out(mem1_cur, HC1, mem1_d)
            emit_out(mem2_cur, HC2, mem2_d)
            emit_out(spk2_fin, HC2, spk2_d)

    nc.compile()
    return nc


_NC_CACHE = {}


def _get_nc():
    key = "full"
    if key not in _NC_CACHE:
        _NC_CACHE[key] = build_snn()
    return _NC_CACHE[key]


def kernel(x, W1, b1, W2, b2):
    """Full-input entry point: shards B across 8 NeuronCores, returns full
    (spk2, mem1, mem2) exactly like reference()."""
    x = np.ascontiguousarray(np.asarray(x, np.float32))
    W1 = np.ascontiguousarray(np.asarray(W1, np.float32))
    b1 = np.ascontiguousarray(np.asarray(b1, np.float32))
    W2 = np.ascontiguousarray(np.asarray(W2, np.float32))
    b2 = np.ascontiguousarray(np.asarray(b2, np.float32))

    nc = _get_nc()
    in_maps = []
    for c in range(N_CORES):
        in_maps.append({
            "x": x[c * BL:(c + 1) * BL],
            "W1": W1, "b1": b1, "W2": W2, "b2": b2,
        })
    res = run_bass_kernel_spmd(nc, in_maps, core_ids=list(range(N_CORES)))
    spk2 = np.concatenate([res.results[c]["spk2"] for c in range(N_CORES)], 0)
    mem1 = np.concatenate([res.results[c]["mem1"] for c in range(N_CORES)], 0)
    mem2 = np.concatenate([res.results[c]["mem2"] for c in range(N_CORES)], 0)
    return spk2, mem1, mem2


# revision 7
# speedup vs baseline: 1.7117x; 1.7117x over previous
"""Feedforward SNN (Linear -> LIF) x2 kernel for Trainium2, 8-core data parallel.

Per-core plan (B sharded 8 ways, BL=32 samples/core):
  - Layer-1 currents for ALL timesteps are computed as fat matmuls (x does
    not depend on recurrent state): Cur1[h1, (t,b)] = W1 @ x^T, fp32.
  - LIF-1 scan over t on [128, HC1*32] tiles (partition = h1 % 128, free =
    (h1chunk, b)); 3 fused DVE ops/step (scalar_tensor_tensor).
  - Spikes are {0,1} == exactly representable in bf16, and W2 is split
    host-side into W2h + W2l (two bf16 terms, Dekker-style, ~exact to fp32):
    layer-2 currents run as 2x bf16 matmuls accumulated in fp32 PSUM --
    2x faster than fp32 matmul with error at the fp32-reorder noise level
    (validated against the reference envelope).
  - LIF-2 scan likewise (2 DVE ops/step; spikes only materialized at t=63).
  - W1^T is PE-transposed once into DRAM scratch and streamed per t-block;
    W2h/W2l are xbar-DMA-transposed (2-byte path) once into scratch.
"""

import os
import sys

import numpy as np

for _p in ("/opt/trn_rl_repo", "/root/.axon_site/_ro/trn_rl_repo"):
    if os.path.isdir(_p) and _p not in sys.path:
        sys.path.insert(0, _p)

import ml_dtypes  # noqa: E402

import concourse.bass as bass  # noqa: E402
import concourse.mybir as mybir  # noqa: E402
import concourse.tile as tile  # noqa: E402
from concourse import bacc  # noqa: E402
from concourse.bass_utils import run_bass_kernel_spmd  # noqa: E402
from concourse.masks import make_identity  # noqa: E402

F32 = mybir.dt.float32
BF16 = mybir.dt.bfloat16
ALU = mybir.AluOpType
AF = mybir.ActivationFunctionType

BETA = 0.9
THR = 1.0

B_FULL, T_FULL, D_FULL, H1_FULL, H2_FULL = 256, 64, 1024, 2048, 2048
N_CORES = 8
BL = B_FULL // N_CORES  # 32


def build_snn(T=T_FULL, D=D_FULL, H1=H1_FULL, H2=H2_FULL, T_NB=16):
    """Build the single-core Bass program (identical across the 8 cores)."""
    P = 128
    KC1 = D // P
    HC1 = H1 // P
    HC2 = H2 // P
    NNB = T // T_NB
    SUB = min(4, T_NB)
    NSUB = T_NB // SUB
    TG = min(4, T_NB)
    NTG = T_NB // TG
    MCQ = min(4, HC2)
    HCQ = min(4, HC1)
    NB32 = T_NB * 32          # matmul free dim per t-block

    assert T % T_NB == 0 and T_NB % SUB == 0 and TG * 32 == 128
    assert HC2 % MCQ == 0 and HC1 % HCQ == 0

    nc = bacc.Bacc("TRN2", target_bir_lowering=False, debug=False)

    x_d = nc.dram_tensor("x", [BL, T, D], F32, kind="ExternalInput")
    w1_d = nc.dram_tensor("W1", [H1, D], F32, kind="ExternalInput")
    b1_d = nc.dram_tensor("b1", [H1], F32, kind="ExternalInput")
    w2h_d = nc.dram_tensor("W2h", [H2, H1], BF16, kind="ExternalInput")
    w2l_d = nc.dram_tensor("W2l", [H2, H1], BF16, kind="ExternalInput")
    b2_d = nc.dram_tensor("b2", [H2], F32, kind="ExternalInput")

    spk2_d = nc.dram_tensor("spk2", [BL, H2], F32, kind="ExternalOutput")
    mem1_d = nc.dram_tensor("mem1", [BL, H1], F32, kind="ExternalOutput")
    mem2_d = nc.dram_tensor("mem2", [BL, H2], F32, kind="ExternalOutput")

    # transposed-weight scratch in DRAM
    w1t_d = nc.dram_tensor("w1t_scr", [KC1, P, H1], F32, kind="Internal")
    w2ht_d = nc.dram_tensor("w2ht_scr", [HC1, P, H2], BF16, kind="Internal")
    w2lt_d = nc.dram_tensor("w2lt_scr", [HC1, P, H2], BF16, kind="Internal")

    x_ap = x_d.ap()
    w1_ap = w1_d.ap()

    with tile.TileContext(nc) as tc:
        from contextlib import ExitStack
        ctx = ExitStack()
        with ctx:
            const = ctx.enter_context(tc.tile_pool(name="const", bufs=1))
            wstage = ctx.enter_context(tc.tile_pool(name="wstage", bufs=2))
            small = ctx.enter_context(tc.tile_pool(name="small", bufs=4))
            xstage = ctx.enter_context(tc.tile_pool(name="xstage", bufs=3))
            xtp = ctx.enter_context(tc.tile_pool(name="xtp", bufs=2))
            w1tp = ctx.enter_context(tc.tile_pool(name="w1tp", bufs=4))
            w2tp = ctx.enter_context(tc.tile_pool(name="w2tp", bufs=3))
            curp = ctx.enter_context(tc.tile_pool(name="curp", bufs=5))
            spk1p = ctx.enter_context(tc.tile_pool(name="spk1p", bufs=1))
            statep = ctx.enter_context(tc.tile_pool(name="statep", bufs=2))
            negzp = ctx.enter_context(tc.tile_pool(name="negzp", bufs=1))
            outp = ctx.enter_context(tc.tile_pool(name="outp", bufs=4))
            tpsum = ctx.enter_context(
                tc.tile_pool(name="tpsum", bufs=2, space="PSUM"))
            mpsum = ctx.enter_context(
                tc.tile_pool(name="mpsum", bufs=6, space="PSUM"))

            ident = const.tile([P, P], F32, name="ident")
            make_identity(nc, ident)

            b1s = const.tile([P, HC1], F32, name="b1s")
            nc.sync.dma_start(b1s[:], b1_d.ap().rearrange("(c p) -> p c", p=P))
            b2s = const.tile([P, HC2], F32, name="b2s")
            nc.sync.dma_start(b2s[:], b2_d.ap().rearrange("(c p) -> p c", p=P))

            # ---------------- phase 0a: W1 -> W1T scratch (PE transpose) ----
            cp_i = 0
            for hc in range(HC1):
                st = wstage.tile([P, KC1 * P], F32, tag="wstage", name="w1st")
                nc.sync.dma_start(st[:], w1_ap[hc * P:(hc + 1) * P, :])
                for k in range(KC1):
                    ps = tpsum.tile([P, P], F32, tag="tp", name="w1ps")
                    nc.tensor.transpose(
                        ps[:], st[:, k * P:(k + 1) * P], ident[:])
                    sb = small.tile([P, P], F32, tag="w1cp", name="w1cp")
                    if cp_i % 2 == 0:
                        nc.scalar.activation(sb[:], ps[:], AF.Copy)
                    else:
                        nc.vector.tensor_copy(sb[:], ps[:])
                    cp_i += 1
                    nc.sync.dma_start(
                        w1t_d.ap()[k, :, hc * P:(hc + 1) * P], sb[:])

            # ---------------- phase 0b: W2h/W2l -> scratch (xbar transp) ----
            for w_ap, scr in ((w2h_d.ap(), w2ht_d), (w2l_d.ap(), w2lt_d)):
                for kc in range(HC1):
                    stg = wstage.tile([P, H2], BF16, tag="wstage", name="w2st")
                    nc.sync.dma_start_transpose(
                        stg[:], w_ap[:, kc * P:(kc + 1) * P])
                    nc.sync.dma_start(scr.ap()[kc], stg[:])

            # ---------------- initial LIF state ----------------------------
            mem1_cur = statep.tile([P, HC1, 32], F32, tag="mem1", name="mem1_0")
            nc.vector.memset(mem1_cur[:], 0.0)
            mem2_cur = statep.tile([P, HC2, 32], F32, tag="mem2", name="mem2_0")
            nc.vector.memset(mem2_cur[:], 0.0)
            spk2_fin = const.tile([P, HC2, 32], F32, name="spk2_fin")

            # ---------------- main t-block pipeline -------------------------
            for nb in range(NNB):
                t0 = nb * T_NB

                # -- x load + PE transpose: xT[p=d%128, kc, (t,b)] -----------
                xt = xtp.tile([P, KC1, NB32], F32, tag="xt", name="xt")
                for tg in range(NTG):
                    xs = xstage.tile([P, D], F32, tag="xs", name="xs")
                    for tr in range(TG):
                        t = t0 + tg * TG + tr
                        nc.sync.dma_start(
                            xs[tr * 32:(tr + 1) * 32, :], x_ap[:, t, :])
                    for kc in range(KC1):
                        ps = tpsum.tile([P, P], F32, tag="tp", name="xtps")
                        nc.tensor.transpose(
                            ps[:], xs[:, kc * P:(kc + 1) * P], ident[:])
                        dst = xt[:, kc, tg * P:(tg + 1) * P]
                        if cp_i % 2 == 0:
                            nc.scalar.activation(dst, ps[:], AF.Copy)
                        else:
                            nc.vector.tensor_copy(dst, ps[:])
                        cp_i += 1

                # -- matmul1 (fp32): cur1[(t,hc,b)] = W1 @ x^T + b1 ----------
                cur1_subs = [curp.tile([P, SUB, HC1, 32], F32, tag="cur1",
                                       name="cur1") for _ in range(NSUB)]
                for hq in range(HC1 // HCQ):
                    pss = [mpsum.tile([P, NB32], F32, tag="mm", name="mm1ps")
                           for _ in range(HCQ)]
                    for kc in range(KC1):
                        w1tt = w1tp.tile([P, HCQ * P], F32, tag="w1t",
                                         name="w1tt")
                        nc.sync.dma_start(
                            w1tt[:],
                            w1t_d.ap()[kc, :, hq * HCQ * P:(hq + 1) * HCQ * P])
                        rhs = xt[:, kc, :]
                        for i in range(HCQ):
                            nc.tensor.matmul(
                                pss[i][:], w1tt[:, i * P:(i + 1) * P], rhs,
                                start=(kc == 0), stop=(kc == KC1 - 1))
                    for i in range(HCQ):
                        hc = hq * HCQ + i
                        psv = pss[i].rearrange("p (t b) -> p t b", b=32)
                        for s in range(NSUB):
                            nc.scalar.activation(
                                cur1_subs[s][:, :, hc, :],
                                psv[:, s * SUB:(s + 1) * SUB, :],
                                AF.Identity, bias=b1s[:, hc:hc + 1])

                # -- scan1 (T_NB steps); spikes (bf16) into spk1[(kc,t,b)] ---
                spk1 = spk1p.tile([P, HC1, NB32], BF16, tag="spk1",
                                  name="spk1")
                for tr in range(T_NB):
                    cur_t = cur1_subs[tr // SUB][:, tr % SUB]  # [P, HC1, 32]
                    negz = negzp.tile([P, HC1, 32], F32, tag="negz1",
                                      name="negz1")
                    nc.vector.scalar_tensor_tensor(
                        negz[:], mem1_cur[:], THR, cur_t,
                        ALU.is_gt, ALU.subtract)
                    mem1_new = statep.tile([P, HC1, 32], F32, tag="mem1",
                                           name="mem1")
                    nc.vector.scalar_tensor_tensor(
                        mem1_new[:], mem1_cur[:], BETA, negz[:],
                        ALU.mult, ALU.subtract)
                    mem1_cur = mem1_new
                    nc.vector.tensor_scalar(
                        spk1[:, :, tr * 32:(tr + 1) * 32], mem1_cur[:],
                        THR, None, ALU.is_gt)

                # -- matmul2 (2x bf16): cur2[(t,mc,b)] = W2 @ spk1^T + b2 ----
                cur2_subs = [curp.tile([P, SUB, HC2, 32], F32, tag="cur2",
                                       name="cur2") for _ in range(NSUB)]
                for mq in range(HC2 // MCQ):
                    pss = [mpsum.tile([P, NB32], F32, tag="mm", name="mm2ps")
                           for _ in range(MCQ)]
                    for kc in range(HC1):
                        wh = w2tp.tile([P, MCQ * P], BF16, tag="w2h",
                                       name="w2h")
                        nc.sync.dma_start(
                            wh[:],
                            w2ht_d.ap()[kc, :, mq * MCQ * P:(mq + 1) * MCQ * P])
                        wl = w2tp.tile([P, MCQ * P], BF16, tag="w2l",
                                       name="w2l")
                        nc.sync.dma_start(
                            wl[:],
                            w2lt_d.ap()[kc, :, mq * MCQ * P:(mq + 1) * MCQ * P])
                        rhs = spk1[:, kc, :]
                        for i in range(MCQ):
                            nc.tensor.matmul(
                                pss[i][:], wh[:, i * P:(i + 1) * P], rhs,
                                start=(kc == 0), stop=False)
                            nc.tensor.matmul(
                                pss[i][:], wl[:, i * P:(i + 1) * P], rhs,
                                start=False, stop=(kc == HC1 - 1))
                    for i in range(MCQ):
                        mc = mq * MCQ + i
                        psv = pss[i].rearrange("p (t b) -> p t b", b=32)
                        for s in range(NSUB):
                            nc.scalar.activation(
                                cur2_subs[s][:, :, mc, :],
                                psv[:, s * SUB:(s + 1) * SUB, :],
                                AF.Identity, bias=b2s[:, mc:mc + 1])

                # -- scan2 (T_NB steps) --------------------------------------
                for tr in range(T_NB):
                    t = t0 + tr
                    cur_t = cur2_subs[tr // SUB][:, tr % SUB]
                    negz = negzp.tile([P, HC2, 32], F32, tag="negz2",
                                      name="negz2")
                    nc.vector.scalar_tensor_tensor(
                        negz[:], mem2_cur[:], THR, cur_t,
                        ALU.is_gt, ALU.subtract)
                    mem2_new = statep.tile([P, HC2, 32], F32, tag="mem2",
                                           name="mem2")
                    nc.vector.scalar_tensor_tensor(
                        mem2_new[:], mem2_cur[:], BETA, negz[:],
                        ALU.mult, ALU.subtract)
                    mem2_cur = mem2_new
                    if t == T - 1:
                        nc.vector.tensor_scalar(
                            spk2_fin[:], mem2_cur[:], THR, None, ALU.is_gt)

            # ---------------- outputs: unscramble [p,(hc,b)] -> [b,h] -------
            def emit_out(state, nch, out_d):
                for hc in range(nch):
                    ps = tpsum.tile([32, P], F32, tag="tp", name="ops")
                    nc.tensor.transpose(ps[:], state[:, hc, :], ident[:])
                    sb = outp.tile([32, P], F32, tag="osb", name="osb")
                    nc.scalar.activation(sb[:], ps[:], AF.Copy)
                    nc.sync.dma_start(
                        out_d.ap()[:, hc * P:(hc + 1) * P], sb[:])

            emit_out(mem1_cur, HC1, mem1_d)
            emit_out(mem2_cur, HC2, mem2_d)
            emit_out(spk2_fin, HC2, spk2_d)

    nc.compile()
    return nc


_NC_CACHE = {}


def _get_nc():
    if "full" not in _NC_CACHE:
        _NC_CACHE["full"] = build_snn()
    return _NC_CACHE["full"]


def split_w2(W2):
    """Dekker-style 2-term bf16 split: W2 ~= W2h + W2l (exact to ~2^-17)."""
    W2 = np.asarray(W2, np.float32)
    W2h = W2.astype(ml_dtypes.bfloat16)
    W2l = (W2 - W2h.astype(np.float32)).astype(ml_dtypes.bfloat16)
    return W2h, W2l


def kernel(x, W1, b1, W2, b2):
    """Full-input entry point: shards B across 8 NeuronCores, returns full
    (spk2, mem1, mem2) exactly like reference()."""
    x = np.ascontiguousarray(np.asarray(x, np.float32))
    W1 = np.ascontiguousarray(np.asarray(W1, np.float32))
    b1 = np.ascontiguousarray(np.asarray(b1, np.float32))
    b2 = np.ascontiguousarray(np.asarray(b2, np.float32))
    W2h, W2l = split_w2(W2)

    nc = _get_nc()
    in_maps = []
    for c in range(N_CORES):
        in_maps.append({
            "x": x[c * BL:(c + 1) * BL],
            "W1": W1, "b1": b1, "W2h": W2h, "W2l": W2l, "b2": b2,
        })
    res = run_bass_kernel_spmd(nc, in_maps, core_ids=list(range(N_CORES)))
    spk2 = np.concatenate([res.results[c]["spk2"] for c in range(N_CORES)], 0)
    mem1 = np.concatenate([res.results[c]["mem1"] for c in range(N_CORES)], 0)
    mem2 = np.concatenate([res.results[c]["mem2"] for c in range(N_CORES)], 0)
    return spk2, mem1, mem2


# revision 8
# speedup vs baseline: 1.7458x; 1.0199x over previous
"""Feedforward SNN (Linear -> LIF) x2 kernel for Trainium2, 8-core data parallel.

Per-core plan (B sharded 8 ways, BL=32 samples/core):
  - Layer-1 currents for ALL timesteps are computed as fat matmuls (x does
    not depend on recurrent state): Cur1[h1, (t,b)] = W1 @ x^T, fp32.
  - LIF-1 scan over t on [128, HC1*32] tiles (partition = h1 % 128, free =
    (h1chunk, b)); 3 fused DVE ops/step (scalar_tensor_tensor).
  - Spikes are {0,1} == exactly representable in bf16, and W2 is split
    host-side into W2h + W2l (two bf16 terms, Dekker-style, ~exact to fp32):
    layer-2 currents run as 2x bf16 matmuls accumulated in fp32 PSUM --
    2x faster than fp32 matmul with error at the fp32-reorder noise level
    (validated against the reference envelope).
  - LIF-2 scan likewise (2 DVE ops/step; spikes only materialized at t=63).
  - W1^T is PE-transposed once into DRAM scratch and streamed per t-block;
    W2h/W2l are xbar-DMA-transposed (2-byte path) once into scratch.
"""

import os
import sys

import numpy as np

for _p in ("/opt/trn_rl_repo", "/root/.axon_site/_ro/trn_rl_repo"):
    if os.path.isdir(_p) and _p not in sys.path:
        sys.path.insert(0, _p)

import ml_dtypes  # noqa: E402

import concourse.bass as bass  # noqa: E402
import concourse.mybir as mybir  # noqa: E402
import concourse.tile as tile  # noqa: E402
from concourse import bacc  # noqa: E402
from concourse.bass_utils import run_bass_kernel_spmd  # noqa: E402
from concourse.masks import make_identity  # noqa: E402

F32 = mybir.dt.float32
BF16 = mybir.dt.bfloat16
ALU = mybir.AluOpType
AF = mybir.ActivationFunctionType

BETA = 0.9
THR = 1.0

B_FULL, T_FULL, D_FULL, H1_FULL, H2_FULL = 256, 64, 1024, 2048, 2048
N_CORES = 8
BL = B_FULL // N_CORES  # 32


def build_snn(T=T_FULL, D=D_FULL, H1=H1_FULL, H2=H2_FULL, T_NB=16):
    """Build the single-core Bass program (identical across the 8 cores)."""
    P = 128
    KC1 = D // P
    HC1 = H1 // P
    HC2 = H2 // P
    NNB = T // T_NB
    SUB = min(4, T_NB)
    NSUB = T_NB // SUB
    TG = min(4, T_NB)
    NTG = T_NB // TG
    MCQ = min(4, HC2)
    HCQ = min(4, HC1)
    NB32 = T_NB * 32          # matmul free dim per t-block

    assert T % T_NB == 0 and T_NB % SUB == 0 and TG * 32 == 128
    assert HC2 % MCQ == 0 and HC1 % HCQ == 0

    nc = bacc.Bacc("TRN2", target_bir_lowering=False, debug=False)

    x_d = nc.dram_tensor("x", [BL, T, D], F32, kind="ExternalInput")
    w1_d = nc.dram_tensor("W1", [H1, D], F32, kind="ExternalInput")
    b1_d = nc.dram_tensor("b1", [H1], F32, kind="ExternalInput")
    w2h_d = nc.dram_tensor("W2h", [H2, H1], BF16, kind="ExternalInput")
    w2l_d = nc.dram_tensor("W2l", [H2, H1], BF16, kind="ExternalInput")
    b2_d = nc.dram_tensor("b2", [H2], F32, kind="ExternalInput")

    spk2_d = nc.dram_tensor("spk2", [BL, H2], F32, kind="ExternalOutput")
    mem1_d = nc.dram_tensor("mem1", [BL, H1], F32, kind="ExternalOutput")
    mem2_d = nc.dram_tensor("mem2", [BL, H2], F32, kind="ExternalOutput")

    # transposed-weight scratch in DRAM
    w1t_d = nc.dram_tensor("w1t_scr", [KC1, P, H1], F32, kind="Internal")
    w2t_d = nc.dram_tensor("w2t_scr", [HC1, P, 2, H2], BF16, kind="Internal")

    x_ap = x_d.ap()
    w1_ap = w1_d.ap()

    with tile.TileContext(nc) as tc:
        from contextlib import ExitStack
        ctx = ExitStack()
        with ctx:
            const = ctx.enter_context(tc.tile_pool(name="const", bufs=1))
            wstage = ctx.enter_context(tc.tile_pool(name="wstage", bufs=2))
            small = ctx.enter_context(tc.tile_pool(name="small", bufs=4))
            xstage = ctx.enter_context(tc.tile_pool(name="xstage", bufs=3))
            xtp = ctx.enter_context(tc.tile_pool(name="xtp", bufs=2))
            w1tp = ctx.enter_context(tc.tile_pool(name="w1tp", bufs=4))
            w2tp = ctx.enter_context(tc.tile_pool(name="w2tp", bufs=4))
            curp = ctx.enter_context(tc.tile_pool(name="curp", bufs=5))
            spk1p = ctx.enter_context(tc.tile_pool(name="spk1p", bufs=1))
            statep = ctx.enter_context(tc.tile_pool(name="statep", bufs=2))
            negzp = ctx.enter_context(tc.tile_pool(name="negzp", bufs=1))
            outp = ctx.enter_context(tc.tile_pool(name="outp", bufs=4))
            tpsum = ctx.enter_context(
                tc.tile_pool(name="tpsum", bufs=2, space="PSUM"))
            mpsum = ctx.enter_context(
                tc.tile_pool(name="mpsum", bufs=6, space="PSUM"))

            ident = const.tile([P, P], F32, name="ident")
            make_identity(nc, ident)

            b1s = const.tile([P, HC1], F32, name="b1s")
            nc.sync.dma_start(b1s[:], b1_d.ap().rearrange("(c p) -> p c", p=P))
            b2s = const.tile([P, HC2], F32, name="b2s")
            nc.sync.dma_start(b2s[:], b2_d.ap().rearrange("(c p) -> p c", p=P))

            # ---------------- phase 0a: W1 -> W1T scratch (PE transpose) ----
            cp_i = 0
            for hc in range(HC1):
                st = wstage.tile([P, KC1 * P], F32, tag="wstage", name="w1st")
                nc.sync.dma_start(st[:], w1_ap[hc * P:(hc + 1) * P, :])
                for k in range(KC1):
                    ps = tpsum.tile([P, P], F32, tag="tp", name="w1ps")
                    nc.tensor.transpose(
                        ps[:], st[:, k * P:(k + 1) * P], ident[:])
                    sb = small.tile([P, P], F32, tag="w1cp", name="w1cp")
                    if cp_i % 2 == 0:
                        nc.scalar.activation(sb[:], ps[:], AF.Copy)
                    else:
                        nc.vector.tensor_copy(sb[:], ps[:])
                    cp_i += 1
                    nc.sync.dma_start(
                        w1t_d.ap()[k, :, hc * P:(hc + 1) * P], sb[:])

            # ---------------- phase 0b: W2h/W2l -> scratch (xbar transp) ----
            for term, w_ap in ((0, w2h_d.ap()), (1, w2l_d.ap())):
                for kc in range(HC1):
                    stg = wstage.tile([P, H2], BF16, tag="wstage", name="w2st")
                    nc.sync.dma_start_transpose(
                        stg[:], w_ap[:, kc * P:(kc + 1) * P])
                    nc.sync.dma_start(w2t_d.ap()[kc, :, term, :], stg[:])

            # ---------------- initial LIF state ----------------------------
            mem1_cur = statep.tile([P, HC1, 32], F32, tag="mem1", name="mem1_0")
            nc.vector.memset(mem1_cur[:], 0.0)
            mem2_cur = statep.tile([P, HC2, 32], F32, tag="mem2", name="mem2_0")
            nc.vector.memset(mem2_cur[:], 0.0)
            spk2_fin = const.tile([P, HC2, 32], F32, name="spk2_fin")

            # ---------------- main t-block pipeline -------------------------
            for nb in range(NNB):
                t0 = nb * T_NB

                # -- x load + PE transpose: xT[p=d%128, kc, (t,b)] -----------
                xt = xtp.tile([P, KC1, NB32], F32, tag="xt", name="xt")
                for tg in range(NTG):
                    xs = xstage.tile([P, D], F32, tag="xs", name="xs")
                    for tr in range(TG):
                        t = t0 + tg * TG + tr
                        nc.gpsimd.dma_start(
                            xs[tr * 32:(tr + 1) * 32, :], x_ap[:, t, :])
                    for kc in range(KC1):
                        ps = tpsum.tile([P, P], F32, tag="tp", name="xtps")
                        nc.tensor.transpose(
                            ps[:], xs[:, kc * P:(kc + 1) * P], ident[:])
                        dst = xt[:, kc, tg * P:(tg + 1) * P]
                        if cp_i % 2 == 0:
                            nc.scalar.activation(dst, ps[:], AF.Copy)
                        else:
                            nc.vector.tensor_copy(dst, ps[:])
                        cp_i += 1

                # -- matmul1 (fp32): cur1[(t,hc,b)] = W1 @ x^T + b1 ----------
                cur1_subs = [curp.tile([P, SUB, HC1, 32], F32, tag="cur1",
                                       name="cur1") for _ in range(NSUB)]
                for hq in range(HC1 // HCQ):
                    pss = [mpsum.tile([P, NB32], F32, tag="mm", name="mm1ps")
                           for _ in range(HCQ)]
                    for kc in range(KC1):
                        w1tt = w1tp.tile([P, HCQ * P], F32, tag="w1t",
                                         name="w1tt")
                        dq = nc.sync if kc % 2 == 0 else nc.scalar
                        dq.dma_start(
                            w1tt[:],
                            w1t_d.ap()[kc, :, hq * HCQ * P:(hq + 1) * HCQ * P])
                        rhs = xt[:, kc, :]
                        for i in range(HCQ):
                            nc.tensor.matmul(
                                pss[i][:], w1tt[:, i * P:(i + 1) * P], rhs,
                                start=(kc == 0), stop=(kc == KC1 - 1))
                    for i in range(HCQ):
                        hc = hq * HCQ + i
                        psv = pss[i].rearrange("p (t b) -> p t b", b=32)
                        for s in range(NSUB):
                            nc.scalar.activation(
                                cur1_subs[s][:, :, hc, :],
                                psv[:, s * SUB:(s + 1) * SUB, :],
                                AF.Identity, bias=b1s[:, hc:hc + 1])

                # -- scan1 (T_NB steps); spikes (bf16) into spk1[(kc,t,b)] ---
                spk1 = spk1p.tile([P, HC1, NB32], BF16, tag="spk1",
                                  name="spk1")
                for tr in range(T_NB):
                    cur_t = cur1_subs[tr // SUB][:, tr % SUB]  # [P, HC1, 32]
                    negz = negzp.tile([P, HC1, 32], F32, tag="negz1",
                                      name="negz1")
                    nc.vector.scalar_tensor_tensor(
                        negz[:], mem1_cur[:], THR, cur_t,
                        ALU.is_gt, ALU.subtract)
                    mem1_new = statep.tile([P, HC1, 32], F32, tag="mem1",
                                           name="mem1")
                    nc.vector.scalar_tensor_tensor(
                        mem1_new[:], mem1_cur[:], BETA, negz[:],
                        ALU.mult, ALU.subtract)
                    mem1_cur = mem1_new
                    nc.vector.tensor_scalar(
                        spk1[:, :, tr * 32:(tr + 1) * 32], mem1_cur[:],
                        THR, None, ALU.is_gt)

                # -- matmul2 (2x bf16): cur2[(t,mc,b)] = W2 @ spk1^T + b2 ----
                cur2_subs = [curp.tile([P, SUB, HC2, 32], F32, tag="cur2",
                                       name="cur2") for _ in range(NSUB)]
                for mq in range(HC2 // MCQ):
                    pss = [mpsum.tile([P, NB32], F32, tag="mm", name="mm2ps")
                           for _ in range(MCQ)]
                    for kc in range(HC1):
                        wt = w2tp.tile([P, 2, MCQ * P], BF16, tag="w2t",
                                       name="w2t")
                        dq = nc.sync if kc % 2 == 0 else nc.scalar
                        dq.dma_start(
                            wt[:],
                            w2t_d.ap()[kc, :, :,
                                       mq * MCQ * P:(mq + 1) * MCQ * P])
                        rhs = spk1[:, kc, :]
                        for i in range(MCQ):
                            nc.tensor.matmul(
                                pss[i][:], wt[:, 0, i * P:(i + 1) * P], rhs,
                                start=(kc == 0), stop=False)
                            nc.tensor.matmul(
                                pss[i][:], wt[:, 1, i * P:(i + 1) * P], rhs,
                                start=False, stop=(kc == HC1 - 1))
                    for i in range(MCQ):
                        mc = mq * MCQ + i
                        psv = pss[i].rearrange("p (t b) -> p t b", b=32)
                        for s in range(NSUB):
                            nc.scalar.activation(
                                cur2_subs[s][:, :, mc, :],
                                psv[:, s * SUB:(s + 1) * SUB, :],
                                AF.Identity, bias=b2s[:, mc:mc + 1])

                # -- scan2 (T_NB steps) --------------------------------------
                for tr in range(T_NB):
                    t = t0 + tr
                    cur_t = cur2_subs[tr // SUB][:, tr % SUB]
                    negz = negzp.tile([P, HC2, 32], F32, tag="negz2",
                                      name="negz2")
                    nc.vector.scalar_tensor_tensor(
                        negz[:], mem2_cur[:], THR, cur_t,
                        ALU.is_gt, ALU.subtract)
                    mem2_new = statep.tile([P, HC2, 32], F32, tag="mem2",
                                           name="mem2")
                    nc.vector.scalar_tensor_tensor(
                        mem2_new[:], mem2_cur[:], BETA, negz[:],
                        ALU.mult, ALU.subtract)
                    mem2_cur = mem2_new
                    if t == T - 1:
                        nc.vector.tensor_scalar(
                            spk2_fin[:], mem2_cur[:], THR, None, ALU.is_gt)

            # ---------------- outputs: unscramble [p,(hc,b)] -> [b,h] -------
            def emit_out(state, nch, out_d):
                for hc in range(nch):
                    ps = tpsum.tile([32, P], F32, tag="tp", name="ops")
                    nc.tensor.transpose(ps[:], state[:, hc, :], ident[:])
                    sb = outp.tile([32, P], F32, tag="osb", name="osb")
                    nc.scalar.activation(sb[:], ps[:], AF.Copy)
                    nc.sync.dma_start(
                        out_d.ap()[:, hc * P:(hc + 1) * P], sb[:])

            emit_out(mem1_cur, HC1, mem1_d)
            emit_out(mem2_cur, HC2, mem2_d)
            emit_out(spk2_fin, HC2, spk2_d)

    nc.compile()
    return nc


_NC_CACHE = {}


def _get_nc():
    if "full" not in _NC_CACHE:
        _NC_CACHE["full"] = build_snn()
    return _NC_CACHE["full"]


def split_w2(W2):
    """Dekker-style 2-term bf16 split: W2 ~= W2h + W2l (exact to ~2^-17)."""
    W2 = np.asarray(W2, np.float32)
    W2h = W2.astype(ml_dtypes.bfloat16)
    W2l = (W2 - W2h.astype(np.float32)).astype(ml_dtypes.bfloat16)
    return W2h, W2l


def kernel(x, W1, b1, W2, b2):
    """Full-input entry point: shards B across 8 NeuronCores, returns full
    (spk2, mem1, mem2) exactly like reference()."""
    x = np.ascontiguousarray(np.asarray(x, np.float32))
    W1 = np.ascontiguousarray(np.asarray(W1, np.float32))
    b1 = np.ascontiguousarray(np.asarray(b1, np.float32))
    b2 = np.ascontiguousarray(np.asarray(b2, np.float32))
    W2h, W2l = split_w2(W2)

    nc = _get_nc()
    in_maps = []
    for c in range(N_CORES):
        in_maps.append({
            "x": x[c * BL:(c + 1) * BL],
            "W1": W1, "b1": b1, "W2h": W2h, "W2l": W2l, "b2": b2,
        })
    res = run_bass_kernel_spmd(nc, in_maps, core_ids=list(range(N_CORES)))
    spk2 = np.concatenate([res.results[c]["spk2"] for c in range(N_CORES)], 0)
    mem1 = np.concatenate([res.results[c]["mem1"] for c in range(N_CORES)], 0)
    mem2 = np.concatenate([res.results[c]["mem2"] for c in range(N_CORES)], 0)
    return spk2, mem1, mem2
